# revision 137
# baseline (speedup 1.0000x reference)
"""GQA attention block (QKV proj + RoPE + causal attention + o_proj),
tensor-parallel over heads across 8 TRN2 NeuronCores.

Sharding: core c owns q heads [4c, 4c+4) (512 q dims), kv head c
(128 kv dims), and wo columns [512c, 512c+512). Each core computes a
full-shape partial of the output projection; the host sums the 8
partials (the "all-reduce") and transposes back.

Layout convention on device: activations are kept feature-major
([dim, seq]) so every matmul contracts over the partition axis with
no transposes:
  QT/KT [d, s]  ->  scores^T [ks, qs] = KT_tile^T . QT   (lhsT=KT, rhs=QT)
  softmax over ks = partition axis: exp on ACT, denominator via
  ones-matmul on PE, division folded into the PV output scaling
  PV: OT [dv, qs] = V_nat^T . P                           (lhsT=V, rhs=P)
  o_proj: outT [e, s] = woT^T . OT                        (lhsT=woT, rhs=OT)

Precision plan: the dense GEMMs (QKV proj, o_proj) run as fp8e4
DoubleRow matmuls (2 K-tiles contracted per instruction at 0.5
cycles/row) with a hi/lo residual split of both operands and the
three significant cross terms (hi.hi + lo.hi + hi.lo) accumulated in
fp32 PSUM - ~1.5e-3 relative error at 0.75x the bf16/fp32r cycle
cost. Weights are pre-scaled by 64 (power of two, folded back into
the PSUM->SBUF copy scale) so their hi/lo parts stay in fp8e4 normal
range; the attention output is pre-scaled by 16 (via the den "ones"
stationary = 1/16) for the same reason. q/k/v/P/scores run in bf16
(same PE rate as fp32r, half the SBUF/DMA). Output partials are
stored bf16 and summed on host.

Scheduling: qkv weights arrive host-FUSED ([wq|wk|wv], 768 cols per
ktile) and pre-tiled partition-major, staged one ktile-quad per kg
iteration so x quads never queue behind multi-us weight transfers on
the serial DMA pipe (one HWDGE dispatch costs 625ns).  Chunk 0 is
the DMA-critical window (all weights + its x must stream in), so it
runs two passes: pass A computes only the hh+hl terms (needs just
w-hi and x0, which is kept resident), and the lh term lands in pass
B at chunk 1's head once w-lo has streamed in behind.  Chunk 1
streams normally; chunks 2/3 hold x resident (their DMAs ride the
idle pipe during earlier chunks) and run their output tiles
sequentially so each RoPE chain overlaps the next tile's matmuls.
Attention runs qt descending with a 2-unit score lookahead and
cross-head score+exp pre-issue; the latency-bound qt<=2 iterations
interleave o_proj column-block emissions between units as PE filler
(gated so an o_proj chunk is only emitted after the attention chunk
feeding it is complete, and pulled BEFORE each iteration's ot8 split
since tile deps are program-order semaphore counts), with the
remaining o_proj drained at deeper PSUM rotation afterwards.  The
softmax denominator of full (non-diagonal) score pairs is one fp8
DoubleRow matmul over a DVE/Pool fp8 copy of P (lagged three units
to hide the cast), which requires EXP_BIAS to keep exp outputs
inside fp8e4 range; diagonal tiles exp the raw scores and zero the
causal triangle with a 0/1 bf16 multiply on DVE (2x mode).  The
softmax finalize chain (reciprocal, scale, fp8 hi/lo split) stays
entirely on DVE - same-engine chaining avoids cross-engine semaphore
hops on the o_proj handoff - and the final output tiles store
per-slice with DMA dispatches alternating between the SP and ACT
sequencer queues.
"""

import sys
from contextlib import ExitStack

import numpy as np
import ml_dtypes

for _p in ("/opt/trn_rl_repo", "/opt/trn_rl_repo/concourse"):
    if _p not in sys.path:
        sys.path.insert(0, _p)

import concourse.bacc as bacc
import concourse.bass as bass
import concourse.tile as tile
from concourse import mybir
from concourse.bass_utils import run_bass_kernel_spmd

F32 = mybir.dt.float32
F32R = mybir.dt.float32r
BF16 = mybir.dt.bfloat16
F8 = mybir.dt.float8e4
E4NP = ml_dtypes.float8_e4m3
BF16NP = ml_dtypes.bfloat16
AF = mybir.ActivationFunctionType
DR = mybir.MatmulPerfMode.DoubleRow

DIM = 4096
SEQ = 2048
HD = 128          # head dim
NCORES = 8
HQ = 4            # q heads per core
DQ = HQ * HD      # 512 q dims per core
NKT = DIM // HD   # 32 contraction tiles
NPAIR = NKT // 2  # 16 DoubleRow k-tile pairs
SQT = SEQ // 512  # 4 seq chunks of 512
WCOLS = DQ + 2 * HD  # fused [wq|wk|wv] out-dim columns per ktile
KSL = slice(DQ, DQ + HD)        # wk columns within the fused tensor
VSL = slice(DQ + HD, WCOLS)     # wv columns
INV_SQRT_HD = 1.0 / np.sqrt(np.float32(HD))
EXP_BIAS = -4.0   # constant shift inside exp; cancels in softmax.
                  # -4 keeps exp outputs within fp8e4 normal range
                  # for the DoubleRow denominator path (max logit
                  # ~5.5 -> p <= e^1.5; typical p ~0.02 >> 2^-9)
WSCALE = 64.0     # weight pre-scale so fp8 hi/lo stays in normal range
OTSCALE = 16.0    # attention-output pre-scale for its fp8 hi/lo split

TRACE = False
LAST_RESULT = None

_cache = {}


def _build(mask_mode):
    """mask_mode: 'zeros' | 'causal' | 'general'."""
    nc = bacc.Bacc("TRN2", target_bir_lowering=False)
    xh = nc.dram_tensor("xh", [DIM, SEQ], F8, kind="ExternalInput")
    xl = nc.dram_tensor("xl", [DIM, SEQ], F8, kind="ExternalInput")
    # qkv weights arrive host-fused ([wq|wk|wv] along the out dim, 768
    # cols per ktile) and pre-tiled partition-major: [p, (ktile m)].
    # One DMA instruction then stages all three tensors for a ktile
    # range, keeping the HWDGE dispatch ring (625ns/instruction) off
    # the critical path during the DMA-saturated first chunk.
    wh = nc.dram_tensor("wh", [HD, NKT * WCOLS], F8, kind="ExternalInput")
    wl = nc.dram_tensor("wl", [HD, NKT * WCOLS], F8, kind="ExternalInput")
    woh = nc.dram_tensor("woh", [HD, HQ * DIM], F8, kind="ExternalInput")
    wol = nc.dram_tensor("wol", [HD, HQ * DIM], F8, kind="ExternalInput")
    cs = nc.dram_tensor("cs", [HD, SEQ], BF16, kind="ExternalInput")
    sn = nc.dram_tensor("sn", [HD, SEQ], BF16, kind="ExternalInput")
    psw = nc.dram_tensor("psw", [HD, HD], F32R, kind="ExternalInput")
    idn = nc.dram_tensor("idn", [HD, HD], BF16, kind="ExternalInput")
    mkt = None
    if mask_mode == "causal":
        # 4 relative diagonal-tile masks (pattern repeats for every qt)
        mkt = nc.dram_tensor("mkt", [HD, 4 * 512], BF16, kind="ExternalInput")
    elif mask_mode == "general":
        mkt = nc.dram_tensor("mkt", [SEQ, SEQ], F32, kind="ExternalInput")
    outt = nc.dram_tensor("outt", [DIM, SEQ], BF16, kind="ExternalOutput")

    QSCALE = float(INV_SQRT_HD / WSCALE)
    KSCALE = float(1.0 / WSCALE)
    OSCALE = float(1.0 / (WSCALE * OTSCALE))

    with ExitStack() as ctx:
        tc = ctx.enter_context(tile.TileContext(nc))

        # ---- persistent pools ----
        const = ctx.enter_context(tc.tile_pool(name="const", bufs=1))
        ones_f32 = const.tile([HD, HD], F32, tag="ones32")
        # den is accumulated pre-divided by OTSCALE so inv = OTSCALE/den and
        # the attention output is scaled into fp8-friendly range for the
        # o_proj hi/lo split; the final output copy divides it back out.
        nc.vector.memset(ones_f32[:], 1.0 / OTSCALE)
        ones_sb = const.tile([HD, HD], BF16, tag="ones")
        nc.scalar.activation(ones_sb[:], ones_f32[:], AF.Copy)
        ebias = const.tile([HD, 1], F32, tag="ebias")
        nc.vector.memset(ebias[:], EXP_BIAS)
        ones8 = const.tile([HD, 2, HD], F8, tag="ones8")
        for _u in range(2):
            nc.scalar.activation(ones8[:, _u, :], ones_f32[:], AF.Copy)
        # identity lives in the persistent pool: phase 1 uses it for the
        # v transposes, phase 2 to accumulate diag masks into scores PSUM
        idn_sb = const.tile([HD, HD], BF16, tag="idn")

        qkvpool = ctx.enter_context(tc.tile_pool(name="qkv", bufs=1))
        # per-chunk tiles so attention reads only depend on the chunks they
        # actually touch (no false whole-tile hazards on the last chunk)
        qrope = [[qkvpool.tile([HD, 512], BF16, tag=f"qr{h}_{c}",
                               name=f"qr{h}_{c}") for c in range(SQT)]
                 for h in range(HQ)]
        krope = [qkvpool.tile([HD, 512], BF16, tag=f"kr{c}", name=f"kr{c}")
                 for c in range(SQT)]
        vnat = [qkvpool.tile([HD, 512], BF16, tag=f"vn{c}", name=f"vn{c}")
                for c in range(SQT)]

        def kr_at(kst):
            return krope[kst // 4][:, (kst % 4) * HD:(kst % 4 + 1) * HD]

        def vn_at(kst):
            return vnat[kst // 4][:, (kst % 4) * HD:(kst % 4 + 1) * HD]

        # ---- phase 1: QKV projection (fp8 DoubleRow 3-term) + RoPE ----
        with ExitStack() as p1:
            wpool = p1.enter_context(tc.tile_pool(name="w1", bufs=1))
            w_sb = [wpool.tile([HD, NKT, WCOLS], F8, tag=f"w{t}", name=f"w{t}")
                    for t in range(2)]
            cs_sb = wpool.tile([HD, SEQ], BF16, tag="cs")
            sn_sb = wpool.tile([HD, SEQ], BF16, tag="sn")
            psw_sb = wpool.tile([HD, HD], F32R, tag="psw")

            def _wslice(t, lo, hi):
                # ktiles [lo, hi) of the fused [p, (k m)] weight tensor
                nc.sync.dma_start(
                    w_sb[t][:, lo:hi, :],
                    (wh, wl)[t][:, lo * WCOLS:hi * WCOLS]
                    .rearrange("p (k m) -> p k m", k=hi - lo))

            xpool = p1.enter_context(tc.tile_pool(name="xstream", bufs=3))
            xchpool = p1.enter_context(tc.tile_pool(name="xch", bufs=2))
            rtmp = p1.enter_context(tc.tile_pool(name="rtmp", bufs=2))
            ps1 = p1.enter_context(tc.tile_pool(name="ps1", bufs=1, space="PSUM"))
            ps1q = p1.enter_context(tc.tile_pool(name="ps1q", bufs=4, space="PSUM"))
            ps1m = p1.enter_context(tc.tile_pool(name="ps1m", bufs=1, space="PSUM"))

            TERMS = ((0, 0), (1, 0), (0, 1))
            xch = {}

            def emit_xch_dmas(stc, gs=range(4)):
                # full-chunk x for the sequential chunks, in 8-ktile slices;
                # hi/lo interleaved per group so early ktile pairs become
                # consumable (all three terms) as soon as possible
                sc_ = slice(stc * 512, (stc + 1) * 512)
                if stc not in xch:
                    xch[stc] = [xchpool.tile([HD, NKT, 512], F8, tag=f"xch{t}",
                                             name=f"xch{t}_{stc}")
                                for t in range(2)]
                tiles = xch[stc]
                for g in gs:
                    for t, xd in ((0, xh), (1, xl)):
                        nc.sync.dma_start(
                            tiles[t][:, g * 8:(g + 1) * 8, :],
                            xd[g * 8 * HD:(g + 1) * 8 * HD, sc_]
                            .rearrange("(k p) m -> p k m", p=HD))

            def make_psum():
                pq = [ps1q.tile([HD, 512], F32, tag="pq", name=f"pq{i}")
                      for i in range(HQ)]
                pk = ps1.tile([HD, 512], F32, tag="pk")
                pv = ps1.tile([HD, 512], F32, tag="pv")
                return pq, pk, pv

            def rope_one(src_ps, dst, ss, scale, on_act):
                raw = rtmp.tile([HD, 512], F32R, tag="qraw5", bufs=5)
                if on_act:
                    nc.scalar.activation(raw[:], src_ps[:], AF.Copy,
                                         scale=scale)
                else:
                    nc.vector.tensor_scalar_mul(raw[:], src_ps[:], scale)
                swp = ps1m.tile([HD, 512], F32, tag="psw3", bufs=1)
                nc.tensor.matmul(swp[:], psw_sb[:], raw[:],
                                 start=True, stop=True)
                t1 = rtmp.tile([HD, 512], F32, tag="t1b", bufs=2)
                nc.vector.tensor_mul(t1[:], raw[:], cs_sb[:, ss])
                t2 = rtmp.tile([HD, 512], F32, tag="t2b", bufs=2)
                nc.vector.tensor_mul(t2[:], swp[:], sn_sb[:, ss])
                nc.vector.tensor_add(dst[:], t1[:], t2[:])

            def v_raw(pv, st):
                # v descale + bf16 cast.  st=3 sits on the phase-1 ->
                # attention handoff where ACT is busy with rope raws, so
                # it uses DVE instead.
                vraw = rtmp.tile([HD, 512], BF16, tag="vraw", bufs=1)
                if st < 3:
                    nc.scalar.activation(vraw[:], pv[:], AF.Copy,
                                         scale=KSCALE)
                else:
                    nc.vector.tensor_scalar_mul(vraw[:], pv[:], KSCALE)
                return vraw

            def v_transpose(vraw, st):
                for j in range(4):
                    vt = ps1m.tile([HD, HD], BF16, tag="pvt")
                    nc.tensor.transpose(vt[:],
                                        vraw[:, j * HD:(j + 1) * HD],
                                        idn_sb[:])
                    nc.vector.tensor_copy(
                        vnat[st][:, j * HD:(j + 1) * HD], vt[:])

            def v_block(pv, st):
                v_transpose(v_raw(pv, st), st)

            def rope_chunk(pq, pk, pv, st):
                # all five raw copies first (ACT/DVE alternating), then the
                # five swp matmuls back-to-back, then the DVE mul/add
                # chains: the PE only waits for raw0 instead of serializing
                # behind each tile's raw in turn
                ss = slice(st * 512, (st + 1) * 512)
                srcs = [(pq[0], qrope[0][st], QSCALE), 
                        (pq[1], qrope[1][st], QSCALE),
                        (pq[2], qrope[2][st], QSCALE),
                        (pq[3], qrope[3][st], QSCALE),
                        (pk, krope[st], KSCALE)]
                raws = []
                for n, (ps, dst, scale) in enumerate(srcs):
                    raw = rtmp.tile([HD, 512], F32R, tag="qraw5", bufs=5,
                                    name="qraw")
                    if n % 2 == 1:
                        nc.scalar.activation(raw[:], ps[:], AF.Copy,
                                             scale=scale)
                    else:
                        nc.vector.tensor_scalar_mul(raw[:], ps[:], scale)
                    raws.append(raw)
                for (ps, dst, scale), raw in zip(srcs, raws):
                    swp = ps1m.tile([HD, 512], F32, tag="psw3", bufs=1,
                                    name="psw")
                    nc.tensor.matmul(swp[:], psw_sb[:], raw[:],
                                     start=True, stop=True)
                    t1 = rtmp.tile([HD, 512], F32, tag="t1b", bufs=2)
                    nc.vector.tensor_mul(t1[:], raw[:], cs_sb[:, ss])
                    t2 = rtmp.tile([HD, 512], F32, tag="t2b", bufs=2)
                    nc.vector.tensor_mul(t2[:], swp[:], sn_sb[:, ss])
                    nc.vector.tensor_add(dst[:], t1[:], t2[:])
                v_block(pv, st)

            def six_matmuls(pq, pk, pv, wi, kpair, xap, fl):
                for mt in range(HQ):
                    msl = slice(mt * HD, (mt + 1) * HD)
                    nc.tensor.matmul(pq[mt][:], w_sb[wi][:, kpair, msl],
                                     xap, perf_mode=DR, **fl)
                nc.tensor.matmul(pk[:], w_sb[wi][:, kpair, KSL], xap,
                                 perf_mode=DR, **fl)
                nc.tensor.matmul(pv[:], w_sb[wi][:, kpair, VSL], xap,
                                 perf_mode=DR, **fl)

            # ---- chunk 0, pass A: terms hh + hl only (w-hi, x0-hi/lo).
            # st=0 is the DMA-critical window (all weights + chunk 0 must
            # stream in); deferring the lh term means only w-hi (9.4us)
            # rides the pipe alongside x chunk 0, leaving real slack.
            # w-lo streams in behind and the lh term lands in pass B. ----
            ss0 = slice(0, 512)
            pq0, pk0, pv0 = make_psum()
            # x0h shares the xch hi-tile rotation: its last read (the lh
            # pass, ~36us) is long done before chunk 3's x arrives there
            x0h = xchpool.tile([HD, NKT, 512], F8, tag="xch0", name="x0h")

            def x0h_dma(lo, hi):
                nc.sync.dma_start(
                    x0h[:, lo:hi, :],
                    xh[lo * HD:hi * HD, ss0]
                    .rearrange("(k p) m -> p k m", p=HD))

            # prologue: minimal first dependency (w-hi ktiles 0-2 + x-hi
            # ktiles 0-2) so the first matmul starts ~3us in, then the
            # rest of kg0's x one DMA ahead of the kg loop
            _wslice(0, 0, 2)
            x0h_dma(0, 2)
            xl_q = {}

            def xl_dma(kg):
                xq8l = xpool.tile([HD, 4, 512], F8, tag="xt1", name="xt1")
                nc.sync.dma_start(
                    xq8l[:],
                    xl[kg * 4 * HD:(kg + 1) * 4 * HD, ss0]
                    .rearrange("(k p) m -> p k m", p=HD))
                xl_q[kg] = xq8l

            xl_dma(0)
            x0h_dma(2, 4)
            _wslice(0, 2, 4)
            x0h_dma(4, 8)
            _wslice(0, 4, 8)
            xl_dma(1)
            x0h_dma(8, 12)
            _wslice(0, 8, 12)
            for kg in range(NKT // 4):
                # stay three quads ahead of the PE on x-hi: the PE clears
                # the early p-state ramp by ~6.5us and then outpaces the
                # DMA's steady rate, so the front buffer must be deep
                if kg < 5:
                    xl_dma(kg + 2)
                    x0h_dma(4 * kg + 12, 4 * kg + 16)
                    _wslice(0, 4 * kg + 12, 4 * kg + 16)
                    if kg == 4:
                        xl_dma(7)
                elif kg == 5:
                    _wslice(1, 0, 4)
                elif kg == 6:
                    _wslice(1, 4, 8)
                else:
                    nc.sync.dma_start(psw_sb[:], psw[:])
                    nc.sync.dma_start(idn_sb[:], idn[:])
                    nc.sync.dma_start(cs_sb[:, 0:512], cs[:, 0:512])
                    nc.sync.dma_start(sn_sb[:, 0:512], sn[:, 0:512])
                xq8l = xl_q.pop(kg)
                for j in range(2):
                    pp = kg * 2 + j       # global pair index
                    kpair = slice(2 * pp, 2 * pp + 2)
                    first = (kg == 0 and j == 0)
                    six_matmuls(pq0, pk0, pv0, 0, kpair, x0h[:, kpair, :],
                                dict(start=first, stop=False))
                    six_matmuls(pq0, pk0, pv0, 0, kpair,
                                xq8l[:, 2 * j:2 * j + 2, :],
                                dict(start=False, stop=False))

            # ---- chunk 0, pass B: deferred lh term (w-lo x x0-hi),
            # w-lo quads streaming in two pairs ahead of use ----
            for ppi in range(NPAIR):
                if ppi % 2 == 0 and ppi < 12:
                    _wslice(1, 2 * ppi + 8, 2 * ppi + 12)
                kpair = slice(2 * ppi, 2 * ppi + 2)
                six_matmuls(pq0, pk0, pv0, 1, kpair, x0h[:, kpair, :],
                            dict(start=False, stop=(ppi == NPAIR - 1)))
            rope_chunk(pq0, pk0, pv0, 0)

            # ---- chunk 1: standard 3-term streaming ----
            ss1 = slice(512, 1024)
            pq1, pk1, pv1 = make_psum()
            for kg in range(NKT // 4):
                xq8 = [xpool.tile([HD, 4, 512], F8, tag=f"xt{t}",
                                  name=f"xt{t}") for t in range(2)]
                nc.sync.dma_start(
                    xq8[0][:],
                    xh[kg * 4 * HD:(kg + 1) * 4 * HD, ss1]
                    .rearrange("(k p) m -> p k m", p=HD))
                nc.sync.dma_start(
                    xq8[1][:],
                    xl[kg * 4 * HD:(kg + 1) * 4 * HD, ss1]
                    .rearrange("(k p) m -> p k m", p=HD))
                if kg == 0:
                    nc.sync.dma_start(cs_sb[:, 512:SEQ], cs[:, 512:SEQ])
                elif kg == 1:
                    nc.sync.dma_start(sn_sb[:, 512:SEQ], sn[:, 512:SEQ])
                elif kg == 5:
                    emit_xch_dmas(2)
                for j in range(2):
                    pp = kg * 2 + j
                    kpair = slice(2 * pp, 2 * pp + 2)
                    first = (kg == 0 and j == 0)
                    last = (kg == NKT // 4 - 1 and j == 1)
                    for ti, (wi, xi) in enumerate(TERMS):
                        six_matmuls(pq1, pk1, pv1, wi, kpair,
                                    xq8[xi][:, 2 * j:2 * j + 2, :],
                                    dict(start=(first and ti == 0),
                                         stop=(last and ti == 2)))
            rope_chunk(pq1, pk1, pv1, 1)

            # ---- chunks 2, 3: x resident; each output tile's RoPE chains
            # overlap the next tile's matmuls.  st=3's x arrives spread
            # across st=2's accumulation units. ----
            for st in (2, 3):
                pq, pk, pv = make_psum()
                xt8 = xch.pop(st)

                def seq_accum(ps, msl):
                    for ppi in range(NPAIR):
                        kpair = slice(2 * ppi, 2 * ppi + 2)
                        for ti, (wi, xi) in enumerate(TERMS):
                            nc.tensor.matmul(
                                ps[:], w_sb[wi][:, kpair, msl],
                                xt8[xi][:, kpair, :],
                                perf_mode=DR,
                                start=(ppi == 0 and ti == 0),
                                stop=(ppi == NPAIR - 1 and ti == 2))

                # order: q0 -> v -> q1..q3 -> k.  v's descale/transposes
                # then overlap the q1 accumulation instead of stalling the
                # PE at the phase boundary, and q0's rope (which feeds the
                # first attention unit) still lands first.
                ss = slice(st * 512, (st + 1) * 512)
                seq_accum(pq[0], slice(0, HD))
                if st == 2:
                    emit_xch_dmas(3, gs=(0,))
                rope_one(pq[0], qrope[0][st], ss, QSCALE, True)
                seq_accum(pv, VSL)
                vraw = v_raw(pv, st)
                for mt in range(1, HQ):
                    seq_accum(pq[mt], slice(mt * HD, (mt + 1) * HD))
                    if st == 2:
                        emit_xch_dmas(3, gs=(mt,))
                    if mt == 1 and st == 2:
                        v_transpose(vraw, st)
                    rope_one(pq[mt], qrope[mt][st], ss, QSCALE, mt % 2 == 0)
                seq_accum(pk, KSL)
                if st < 3:
                    rope_one(pk, krope[st], ss, KSCALE, True)
                else:
                    # split the last rope: emit the raw copy first, then
                    # the (ready) v transposes, THEN the swp matmul - the
                    # in-order PE chews the transposes while the raw copy
                    # lands instead of stalling at the phase boundary
                    raw = rtmp.tile([HD, 512], F32R, tag="qraw5", bufs=5)
                    nc.vector.tensor_scalar_mul(raw[:], pk[:], KSCALE)
                    v_transpose(vraw, st)
                    swp = ps1m.tile([HD, 512], F32, tag="psw3", bufs=1)
                    nc.tensor.matmul(swp[:], psw_sb[:], raw[:],
                                     start=True, stop=True)
                    t1 = rtmp.tile([HD, 512], F32, tag="t1b", bufs=2)
                    nc.vector.tensor_mul(t1[:], raw[:], cs_sb[:, ss])
                    t2 = rtmp.tile([HD, 512], F32, tag="t2b", bufs=2)
                    nc.vector.tensor_mul(t2[:], swp[:], sn_sb[:, ss])
                    nc.vector.tensor_add(krope[st][:], t1[:], t2[:])

        # ---- phase 2: attention;  phase 3: output projection ----
        with ExitStack() as p2:
            wopool = p2.enter_context(tc.tile_pool(name="wo", bufs=1))
            wo_sb = [wopool.tile([HD, HQ, DIM], F8, tag=f"wo{t}", name=f"wo{t}")
                     for t in range(2)]
            wo_dma_emitted = [False]

            def emit_wo_dmas():
                if not wo_dma_emitted[0]:
                    wo_dma_emitted[0] = True
                    nc.sync.dma_start(
                        wo_sb[0][:], woh[:].rearrange("p (k m) -> p k m", k=HQ))
                    nc.sync.dma_start(
                        wo_sb[1][:], wol[:].rearrange("p (k m) -> p k m", k=HQ))

            otpool = p2.enter_context(tc.tile_pool(name="ot", bufs=1))
            # attention output per head, fp8 hi/lo split for the o_proj
            ot8 = [otpool.tile([HD, HQ, SEQ], F8, tag=f"ot8{t}", name=f"ot8{t}")
                   for t in range(2)]

            mpool = p2.enter_context(tc.tile_pool(name="mk", bufs=1))
            spool = p2.enter_context(tc.tile_pool(name="sp", bufs=4))

            mk_sb = None
            if mask_mode == "causal":
                mk_sb = mpool.tile([HD, 4, 512], BF16, tag="mkd")
                nc.sync.dma_start(
                    mk_sb[:], mkt[:].rearrange("p (k m) -> p k m", k=4))

            gen_masks = {}

            def emit_gen_masks(qt):
                qs = slice(qt * 512, (qt + 1) * 512)
                out = {}
                for kst in range(16):
                    m = mpool.tile([HD, 512], F32, tag=f"mk{kst}",
                                   name=f"mk{kst}")
                    nc.sync.dma_start(
                        m[:], mkt[kst * HD:(kst + 1) * HD, qs])
                    out[kst] = m
                return out

            def npair_of(qt):
                return 2 * qt if mask_mode == "causal" else 8

            def nunit_of(qt):
                return npair_of(qt) + (4 if mask_mode == "causal" else 0)

            def issue_scores_for(qt, h, i, ps2):
                npair = npair_of(qt)
                qs = slice(qt * 512, (qt + 1) * 512)
                sp = ps2.tile([HD, 1024], F32, tag="pst")
                if i < npair:
                    for u in range(2):
                        kst = 2 * i + u
                        nc.tensor.matmul(
                            sp[:, u * 512:(u + 1) * 512],
                            kr_at(kst),
                            qrope[h][qt][:],
                            start=True, stop=True)
                else:
                    # diagonal tile, columns < c0 fully masked
                    r = i - npair
                    kst = 4 * qt + r
                    c0 = r * HD
                    nc.tensor.matmul(
                        sp[:, c0:512],
                        kr_at(kst),
                        qrope[h][qt][:, c0:512],
                        start=True, stop=True)
                return sp

            def issue_exp_for(qt, i, sp, ppool):
                npair = npair_of(qt)
                pb = ppool.tile([HD, 1024], BF16, tag="pexp")
                if i < npair:
                    if mask_mode == "general":
                        tmp = ppool.tile([HD, 1024], F32, tag="padd", bufs=2)
                        for u in range(2):
                            usl = slice(u * 512, (u + 1) * 512)
                            nc.vector.tensor_add(
                                tmp[:, usl], sp[:, usl],
                                gen_masks[qt][2 * i + u][:])
                        nc.scalar.activation(pb[:], tmp[:], AF.Exp,
                                             bias=ebias[:])
                    else:
                        nc.scalar.activation(pb[:], sp[:], AF.Exp,
                                             bias=ebias[:])
                else:
                    # exp the raw diag scores, then zero the masked
                    # triangle with a 0/1 multiply (bf16 2x mode on DVE
                    # is ~4x cheaper than an f32 mask-add, and the exp of
                    # an unmasked score is bounded by e^(5.5+bias))
                    r = i - npair
                    c0 = r * HD
                    pbr = ppool.tile([HD, 1024], BF16, tag="praw", bufs=2)
                    nc.scalar.activation(pbr[:, c0:512], sp[:, c0:512],
                                         AF.Exp, bias=ebias[:])
                    nc.vector.tensor_mul(pb[:, c0:512], pbr[:, c0:512],
                                         mk_sb[:, r, c0:])
                return pb

            pre_store = {}

            def attn_iter(qt, h, ps2, ps2a, ppool, filler,
                          prescore_next=None, lookahead=2, p8pool=None):
                qs = slice(qt * 512, (qt + 1) * 512)
                npair = npair_of(qt)
                nunit = nunit_of(qt)
                sps = [None] * nunit
                pbs = [None] * nunit

                pre = pre_store.pop((qt, h), None)
                # lead with one diag unit ONLY when prescored: its short
                # exp un-blocks the first PV quickly.  The first (un-
                # prescored) iteration keeps natural order - its diag
                # units need the LAST chunk's k/v, which lands at the
                # very end of phase 1.
                diag_first = (pre is not None and nunit > npair
                              and npair > 0)
                if pre is not None:
                    j0, j1, sps[j0], sps[j1], pbs[j0], pbs[j1] = pre
                    if nunit > 2:
                        # the prescored exps already consumed s0/s1, so
                        # the score pool is empty here: issue a third
                        # unit up front to cover the first PV's exp wait
                        j2 = 1 if diag_first and npair > 1 else 2
                        sps[j2] = issue_scores_for(qt, h, j2, ps2)
                        if diag_first and npair > 2:
                            sps[2] = issue_scores_for(qt, h, 2, ps2)
                else:
                    for j in [0, 1][:min(lookahead, nunit)]:
                        sps[j] = issue_scores_for(qt, h, j, ps2)

                den = ps2a.tile([HD, 512], F32, tag="pden")
                otp = ps2a.tile([HD, 512], F32, tag="pot")
                # den_dr: non-diag pair units compute den as one fp8
                # DoubleRow matmul over a Pool-engine fp8 copy of P (lagged
                # one unit to hide the cast latency)
                den_dr = p8pool is not None and npair > 0
                pend = []
                den_state = [False]

                def den_start():
                    # PSUM accumulate: exactly the FIRST den matmul in
                    # emission (= PE execution) order must carry start=True
                    s = not den_state[0]
                    den_state[0] = True
                    return s

                def flush_den_dr(keep=0, final=False):
                    while len(pend) > keep:
                        p8t = pend.pop(0)
                        nc.tensor.matmul(
                            den[:], ones8[:],
                            p8t[:].rearrange("p (u m) -> p u m", u=2),
                            perf_mode=DR, start=den_start(),
                            stop=(final and not pend))

                otp_state = [False]

                def otp_start():
                    s = not otp_state[0]
                    otp_state[0] = True
                    return s

                order = ([npair] + list(range(npair)) +
                         list(range(npair + 1, nunit))
                         if diag_first else list(range(nunit)))
                for pos, i in enumerate(order):
                    if lookahead + pos < nunit and \
                            sps[order[lookahead + pos]] is None:
                        j = order[lookahead + pos]
                        sps[j] = issue_scores_for(qt, h, j, ps2)
                    if pbs[i] is None:
                        pbs[i] = issue_exp_for(qt, i, sps[i], ppool)
                    fl_last = (pos == nunit - 1)
                    if i < npair:
                        if den_dr:
                            p8t = p8pool.tile([HD, 1024], F8, tag="p8")
                            eng = nc.vector if i != npair - 1 else nc.gpsimd
                            eng.tensor_copy(p8t[:], pbs[i][:])
                        for u in range(2):
                            kst = 2 * i + u
                            pr = pbs[i][:, u * 512:(u + 1) * 512]
                            if not den_dr:
                                nc.tensor.matmul(
                                    den[:], ones_sb[:], pr,
                                    start=den_start(),
                                    stop=(fl_last and u == 1))
                            nc.tensor.matmul(
                                otp[:], vn_at(kst), pr,
                                start=otp_start(),
                                stop=(fl_last and u == 1))
                        if den_dr:
                            flush_den_dr(keep=3)
                            pend.append(p8t)
                    else:
                        r = i - npair
                        kst = 4 * qt + r
                        c0 = r * HD
                        if den_dr:
                            flush_den_dr(keep=max(0, 3 - r))
                        pr = pbs[i][:, c0:512]
                        nc.tensor.matmul(
                            den[:, c0:], ones_sb[:], pr,
                            start=den_start(), stop=fl_last)
                        nc.tensor.matmul(
                            otp[:, c0:], vn_at(kst), pr,
                            start=otp_start(), stop=fl_last)
                    if fl_last and prescore_next is not None:
                        # pre-issue the next iteration's first two score
                        # units AND the first exp, so its exp pipeline
                        # starts before this iteration's DVE drain; doing
                        # it before the final den flush also gives the
                        # last casts time to land
                        qn, hn = prescore_next
                        npn, nun = npair_of(qn), nunit_of(qn)
                        j0, j1 = ((npn, 0) if nun > npn and npn > 0
                                  else (0, 1))
                        s0 = issue_scores_for(qn, hn, j0, ps2)
                        s1 = issue_scores_for(qn, hn, j1, ps2)
                        pre_store[(qn, hn)] = (
                            j0, j1, s0, s1,
                            issue_exp_for(qn, j0, s0, ppool),
                            issue_exp_for(qn, j1, s1, ppool))
                    if filler is not None:
                        filler()
                if den_dr:
                    flush_den_dr(keep=0, final=True)
                inv = spool.tile([HD, 512], F32, tag="inv")
                nc.vector.reciprocal(inv[:], den[:])
                ots = spool.tile([HD, 512], F32, tag="ots")
                nc.vector.tensor_mul(ots[:], otp[:], inv[:])
                # pull filler work BEFORE emitting the ot8 split: tile
                # deps are program-order semaphore counts, so an o_proj
                # unit emitted after this split would stall on it even
                # though it reads other (qt, h) slices
                if filler is not None:
                    filler()
                # fp8 hi/lo split of the attention output; hi cast on the
                # lightly-loaded Pool engine keeps ACT free for exp
                nc.vector.tensor_copy(ot8[0][:, h, qs], ots[:])
                nc.vector.tensor_sub(ot8[1][:, h, qs], ots[:],
                                     ot8[0][:, h, qs])

            # ---- phase 3 emitter: o_proj (fp8 DoubleRow 3-term), one
            # [128,512] column block per generator step so it can be
            # interleaved into the attention tail as PE filler work ----
            OTERMS = ((0, 0), (1, 0), (0, 1))

            def oproj_units(sts, ps3, opool, egs=None):
                for st in sts:
                    ss = slice(st * 512, (st + 1) * 512)
                    for eg in (range(DIM // HD // 4) if egs is None
                               else egs):
                        last_grp = (st == 0 and eg == DIM // HD // 4 - 1)
                        ocp = opool.tile([HD, 4, 512], BF16, tag="ocp")
                        for ej in range(4):
                            et = eg * 4 + ej
                            esl = slice(et * HD, (et + 1) * HD)
                            po = ps3.tile([HD, 512], F32, tag="po")
                            for pi in range(2):
                                hpair = slice(2 * pi, 2 * pi + 2)
                                for ti, (wi, oi) in enumerate(OTERMS):
                                    nc.tensor.matmul(
                                        po[:],
                                        wo_sb[wi][:, hpair, esl],
                                        ot8[oi][:, hpair, ss],
                                        perf_mode=DR,
                                        start=(pi == 0 and ti == 0),
                                        stop=(pi == 1 and ti == 2),
                                    )
                            osl = ocp[:, ej, :]
                            # GPSIMD cannot read PSUM, so the descale+cast
                            # alternates ACT/DVE
                            if ej % 2 == 0:
                                nc.scalar.activation(osl, po[:], AF.Copy,
                                                     scale=OSCALE)
                            else:
                                nc.vector.tensor_scalar_mul(osl, po[:],
                                                            OSCALE)
                            if last_grp:
                                # final tiles: store per-slice, and
                                # alternate the DMA dispatches between the
                                # SP and ACT sequencers so the four 650ns
                                # dispatch slots don't serialize on SP
                                dq = nc.sync if ej % 2 == 0 else nc.scalar
                                dq.dma_start(
                                    outt[et * HD:(et + 1) * HD, ss], osl)
                            yield
                        if not last_grp:
                            nc.sync.dma_start(
                                outt[eg * 4 * HD:(eg + 1) * 4 * HD, ss]
                                .rearrange("(e p) m -> p e m", p=HD),
                                ocp[:])

            if mask_mode == "causal":
                # qt=2 runs FIRST (roughly PE/ACT balanced on its own),
                # so that qt=3 — whose exp volume makes it ACT-bound with
                # ~25% PE slack — runs inside the filler section where
                # o_proj(st=2) units keep the PE busy under the exp chain.
                p8pool = p2.enter_context(tc.tile_pool(name="p8p", bufs=6))
                with ExitStack() as patt:
                    ppool = patt.enter_context(tc.tile_pool(name="pp", bufs=6))
                    ps2 = patt.enter_context(
                        tc.tile_pool(name="ps2", bufs=3, space="PSUM"))
                    ps2a = patt.enter_context(
                        tc.tile_pool(name="ps2a", bufs=1, space="PSUM"))
                    emit_wo_dmas()
                    for h in range(HQ):
                        nxt = (3, h + 1) if h + 1 < HQ else None
                        attn_iter(3, h, ps2, ps2a, ppool, None,
                                  prescore_next=nxt, p8pool=p8pool)
                # filler section: qt=3 (fed by o_proj st=2), then qt=1,
                # then qt=0 (fed by st=3 and st=1)
                with ExitStack() as ptail:
                    ppool2 = ptail.enter_context(
                        tc.tile_pool(name="pp2", bufs=6))
                    ps2t = ptail.enter_context(
                        tc.tile_pool(name="ps2t", bufs=2, space="PSUM"))
                    ps2a2 = ptail.enter_context(
                        tc.tile_pool(name="ps2a2", bufs=1, space="PSUM"))
                    ps3 = ptail.enter_context(
                        tc.tile_pool(name="ps3", bufs=2, space="PSUM"))
                    opool = ptail.enter_context(
                        tc.tile_pool(name="ostage", bufs=4))
                    gen = oproj_units((3, 2), ps3, opool)
                    # st=2 units (32) are ready once qt=2 is done; st=3
                    # units unlock after qt=3's last head completes
                    pulled = [0]
                    limit = [32]
                    _done = object()

                    def filler_gen():
                        if pulled[0] < limit[0]:
                            if next(gen, _done) is not _done:
                                pulled[0] += 1

                    seq = [(qt, h) for qt in (2, 1, 0) for h in range(HQ)]
                    for n, (qt, h) in enumerate(seq[:8]):
                        attn_iter(qt, h, ps2t, ps2a2, ppool2, filler_gen,
                                  prescore_next=seq[n + 1], p8pool=p8pool)
                        if (qt, h) == (2, HQ - 1):
                            limit[0] = 64
                    gen2 = oproj_units((1,), ps3, opool, egs=range(0, 4))

                    def filler_tail():
                        if next(gen, _done) is _done:
                            next(gen2, None)

                    for h in range(HQ):
                        nxt = (0, h + 1) if h + 1 < HQ else None
                        attn_iter(0, h, ps2t, ps2a2, ppool2, filler_tail,
                                  prescore_next=nxt)
                    for _ in gen:
                        pass
                    for _ in gen2:
                        pass
                # bulk o_proj drain with deep PSUM rotation
                with ExitStack() as p3d:
                    ps3d = p3d.enter_context(
                        tc.tile_pool(name="ps3d", bufs=4, space="PSUM"))
                    opool2 = p3d.enter_context(
                        tc.tile_pool(name="ostage2", bufs=3))
                    for _ in oproj_units((1,), ps3d, opool2,
                                         egs=range(4, 8)):
                        pass
                    for _ in oproj_units((0,), ps3d, opool2):
                        pass
            else:
                with ExitStack() as patt:
                    ppool = patt.enter_context(tc.tile_pool(name="pp", bufs=6))
                    ps2 = patt.enter_context(
                        tc.tile_pool(name="ps2", bufs=3, space="PSUM"))
                    ps2a = patt.enter_context(
                        tc.tile_pool(name="ps2a", bufs=1, space="PSUM"))
                    emit_wo_dmas()
                    for qt in range(SQT - 1, -1, -1):
                        if mask_mode == "general" and qt not in gen_masks:
                            gen_masks[qt] = emit_gen_masks(qt)
                        for h in range(HQ):
                            attn_iter(qt, h, ps2, ps2a, ppool, None)
                    pre_store.clear()
                with ExitStack() as p3:
                    ps3 = p3.enter_context(
                        tc.tile_pool(name="ps3", bufs=4, space="PSUM"))
                    opool = p3.enter_context(
                        tc.tile_pool(name="ostage", bufs=3))
                    for _ in oproj_units((3, 2, 1, 0), ps3, opool):
                        pass

    nc.compile()
    return nc


def _split8(a, scale=1.0):
    s = np.clip(a * np.float32(scale), -224.0, 224.0)
    hi = s.astype(E4NP)
    lo = np.clip(s - hi.astype(np.float32), -224.0, 224.0).astype(E4NP)
    return np.ascontiguousarray(hi), np.ascontiguousarray(lo)


def _prep_consts(freqs_cos, freqs_sin):
    cos = np.asarray(freqs_cos, dtype=np.float32)
    sin = np.asarray(freqs_sin, dtype=np.float32)
    C = np.empty((HD, SEQ), np.float32)
    S = np.empty((HD, SEQ), np.float32)
    C[0::2] = cos.T
    C[1::2] = cos.T
    S[0::2] = -sin.T
    S[1::2] = sin.T
    psw = np.zeros((HD, HD), np.float32)
    j = np.arange(0, HD, 2)
    psw[j + 1, j] = 1.0
    psw[j, j + 1] = 1.0
    idn = np.eye(HD, dtype=np.float32).astype(BF16NP)
    return C, S, psw, idn


def _mask_mode(mask):
    if not mask.any():
        return "zeros"
    neg = mask.min()
    tril = np.tril(np.ones((SEQ, SEQ), dtype=bool))
    if neg <= -1e8 and not mask[tril].any() and np.all(mask[~tril] == neg):
        return "causal"
    return "general"


def kernel(x, wq, wk, wv, wo, freqs_cos, freqs_sin, mask, start_pos):
    global LAST_RESULT
    assert int(start_pos) == 0, "kernel hardcodes start_pos=0 (full prefill)"
    x = np.asarray(x, dtype=np.float32)
    wq = np.asarray(wq, dtype=np.float32)
    wk = np.asarray(wk, dtype=np.float32)
    wv = np.asarray(wv, dtype=np.float32)
    wo = np.asarray(wo, dtype=np.float32)
    mask = np.asarray(mask, dtype=np.float32)

    mode = _mask_mode(mask)
    if mode not in _cache:
        _cache[mode] = _build(mode)
    nc = _cache[mode]

    xt = np.ascontiguousarray(x.reshape(SEQ, DIM).T)
    xh8, xl8 = _split8(xt)
    C, S, psw, idn = _prep_consts(freqs_cos, freqs_sin)
    mkt = None
    if mode == "causal":
        # 4 relative diagonal tile masks: tile r is mask.T[r*128:(r+1)*128,
        # 0:512] (the pattern depends only on kst - 4*qt)
        mt = np.ascontiguousarray(mask.T[:512, :512])
        mkt = np.concatenate([mt[r * HD:(r + 1) * HD, :] for r in range(4)],
                             axis=1)
        # 0/1 multiplicative mask (applied to exp'd scores on DVE)
        mkt = (mkt > -1.0).astype(np.float32).astype(BF16NP)
        mkt = np.ascontiguousarray(mkt)
    elif mode == "general":
        mkt = np.ascontiguousarray(mask.T)

    def _ptile(a, m):
        # [DIM_contract, m] -> partition-major [128, (ktile m)]
        k = a.shape[0] // HD
        return np.ascontiguousarray(
            a.reshape(k, HD, m).transpose(1, 0, 2).reshape(HD, k * m))

    in_maps = []
    for c in range(NCORES):
        # fused [wq|wk|wv] weight block for this core: [DIM, 768]
        wfull = np.concatenate([
            wq[c * DQ:(c + 1) * DQ, :].T,
            wk[c * HD:(c + 1) * HD, :].T,
            wv[c * HD:(c + 1) * HD, :].T], axis=1)
        wh8, wl8 = _split8(wfull, WSCALE)
        woh8, wol8 = _split8(wo[:, c * DQ:(c + 1) * DQ].T, WSCALE)
        wh8, wl8 = _ptile(wh8, WCOLS), _ptile(wl8, WCOLS)
        woh8, wol8 = _ptile(woh8, DIM), _ptile(wol8, DIM)
        m = {
            "xh": xh8, "xl": xl8,
            "wh": wh8, "wl": wl8,
            "woh": woh8, "wol": wol8,
            "cs": C.astype(BF16NP), "sn": S.astype(BF16NP),
            "psw": psw, "idn": idn,
        }
        if mkt is not None:
            m["mkt"] = mkt
        in_maps.append(m)

    res = run_bass_kernel_spmd(nc, in_maps, core_ids=list(range(NCORES)),
                               trace=TRACE)
    LAST_RESULT = res
    acc = np.zeros((DIM, SEQ), dtype=np.float64)
    for c in range(NCORES):
        acc += res.results[c]["outt"].astype(np.float64)
    return np.ascontiguousarray(acc.T).astype(np.float32).reshape(1, SEQ, DIM)

